# revision 13
# baseline (speedup 1.0000x reference)
"""Trainium2 Bass kernel for nn_Attention_57423712748130.

Computation (per batch b):
  X4 = x[b] viewed (C=256, N=4096)   [raw reshape]
  Q4 = silu(BN(q_w @ X4))            (256, 4096)
  KV4 = silu(BN(kv_w @ Y4))          (128, 4096)
  q[n,h,d]  = Q4[n1, n0*256+h*64+d]      n = n1*16+n0
  k[m,h,d]  = KV4[m1, m0*512 + h*64+d]   m = m1*8+m0
  v[m,h,d]  = KV4[m1, m0*512+256+h*64+d]
  att = softmax(q k^T / 8); o = att v
  out rows [h*1024,(h+1)*1024) = O_h @ proj_w.T + proj_b
    where O_h[n2, n3*64+d] = o[4*n2+n3, d]

Sharding: 8 cores = (batch b in 0..3) x (head-pair hp in 0..1); each core
computes heads {2hp, 2hp+1} of batch b = rows [hp*2048,(hp+1)*2048) of out[b].

On-core strategy (v3 — ScalarE(exp)-bound; PE kept warm and never on the
critical path):
 - the loop is paced by 64 exp ACTIVATEs of [128,1024] (~1.15us each, the
   hard ScalarE floor); everything else is scheduled to hide behind them
 - ALL input DMAs issue as the first program instructions on the three
   DMA queues (sync/scalar/gpsimd), k-conv inputs first, so the first
   conv starts ~6us earlier than waiting for the full input set
 - PSUM is ONE shared ring: tag "scp" bufs=3 x [128,1024] carries every
   conv/score/warmup psum (scores get a 3-deep ring => produced up to 3
   tiles ahead of exp, absorbing HAM cold-clock phases), tag "av" bufs=2
   x [128,512] carries the av accumulators and the projection psums
 - convs are PAIRED into [128,1024] psums (half the tanh/DVE ops + sems)
 - ~4us of garbage-operand warmup matmuls (no DMA dependency) flip the
   HAM clock gate to 8/8 before the first real matmul
 - scoresT[m,n]: one [128,1024] psum per m0 holds BOTH heads; the two
   score matmuls (PE row groups 0-63/64-127) overlap in the array;
   av interleaves both heads by m0, chasing exp pairwise
 - att@v contracts over m with an extra 2.0-column on V producing
   softmax denominators as psum row 64; normalize = gpsimd
   partition_broadcast STRAIGHT from psum row 64 -> reciprocal_approx_
   fast on the [64,512] broadcast -> multiply folded into the PSUM
   evacuation
 - normalization + per-head projection run per q0-quarter, pipelined
   behind the attention of later quarters; the output DMA un-permutes
   rows; the last quarter splits per half across two queues
"""

import ml_dtypes
import numpy as np

B = 4
N_TOK = 4096
C = 256
BN_EPS = 1e-5

_CACHE = {}


def _build():
    import concourse.bacc as bacc
    import concourse.bass as bass
    import concourse.tile as tile
    from concourse import mybir

    f32 = mybir.dt.float32
    bf16 = mybir.dt.bfloat16
    adt = bf16
    AF = mybir.ActivationFunctionType

    nc = bacc.Bacc("TRN2", target_bir_lowering=False, debug=False, num_devices=8)

    xq = nc.dram_tensor("xq", [128, 4096], bf16, kind="ExternalInput")
    yk = nc.dram_tensor("yk", [128, 2048], bf16, kind="ExternalInput")
    yv = nc.dram_tensor("yv", [128, 2048], bf16, kind="ExternalInput")
    wq = nc.dram_tensor("wq", [128, 512], bf16, kind="ExternalInput")
    wkv = nc.dram_tensor("wkv", [128, 256], bf16, kind="ExternalInput")
    wp = nc.dram_tensor("wp", [128, 512], bf16, kind="ExternalInput")
    biases = nc.dram_tensor("biases", [1, 2176], bf16, kind="ExternalInput")
    bp = nc.dram_tensor("bp", [1, 512], f32, kind="ExternalInput")
    out = nc.dram_tensor("out", [2048, 256], bf16, kind="ExternalOutput")

    with tile.TileContext(nc) as tc:
        with (
            tc.tile_pool(name="const", bufs=1) as cp,
            tc.tile_pool(name="actt", bufs=3) as actt,
            tc.tile_pool(name="attp", bufs=24) as attp,
            tc.tile_pool(name="outp", bufs=3) as outp,
            tc.tile_pool(name="gp", bufs=3) as gp,
            tc.tile_pool(name="ps", bufs=3, space="PSUM") as ps,
        ):
            # ---- all input DMAs first, spread over the 3 DMA queues,
            # k-conv inputs (wkv, yk) with highest priority ----
            xqc = cp.tile([128, 4096], bf16, tag="xq", name="xq")
            ykc = cp.tile([128, 2048], bf16, tag="yk", name="yk")
            yvc = cp.tile([128, 2048], bf16, tag="yv", name="yv")
            wqc = cp.tile([128, 512], bf16, tag="wq", name="wq")
            wkvc = cp.tile([128, 256], bf16, tag="wkv", name="wkv")
            wpc = cp.tile([128, 512], bf16, tag="wp", name="wp")
            bias_sb = cp.tile([1, 2176], bf16, tag="biases", name="bias_sb")
            bp_bc = cp.tile([128, 512], f32, tag="bp_bc", name="bp_bc")

            def piece(eng, dst, src, lo, hi):
                eng.dma_start(dst[:, lo:hi], src.ap()[:, lo:hi])

            # sync: k conv inputs
            piece(nc.sync, wkvc, wkv, 0, 256)
            piece(nc.sync, ykc, yk, 0, 1024)
            piece(nc.sync, ykc, yk, 1024, 2048)
            # scalar: q conv (t2 0-3) inputs, then the ACT-table preload
            piece(nc.scalar, wqc, wq, 0, 512)
            piece(nc.scalar, xqc, xq, 0, 1024)
            piece(nc.scalar, xqc, xq, 2048, 3072)
            # gpsimd: biases tiny-first, v conv, late xq halves, proj w
            piece(nc.gpsimd, bias_sb, biases, 0, 2176)
            piece(nc.gpsimd, yvc, yv, 0, 1024)
            piece(nc.gpsimd, yvc, yv, 1024, 2048)
            piece(nc.gpsimd, xqc, xq, 1024, 2048)
            piece(nc.gpsimd, xqc, xq, 3072, 4096)
            piece(nc.gpsimd, wpc, wp, 0, 512)
            nc.gpsimd.dma_start(bp_bc[:], bp.ap().partition_broadcast(128))

            xq_sb = [xqc[:, i * 2048 : (i + 1) * 2048] for i in range(2)]
            wkv_sb = [wkvc[:, i * 128 : (i + 1) * 128] for i in range(2)]
            yk_sb = [ykc[:, i * 1024 : (i + 1) * 1024] for i in range(2)]
            yv_sb = [yvc[:, i * 1024 : (i + 1) * 1024] for i in range(2)]
            wq_sb = [wqc[:, i * 256 : (i + 1) * 256] for i in range(2)]
            wp_sb = [wpc[:, i * 256 : (i + 1) * 256] for i in range(2)]
            bq4_sb = bias_sb[0:1, 0:1024]
            bkv8_sb = bias_sb[0:1, 1024:2048]
            bkvr_sb = bias_sb[0:1, 2048:2176]

            ones_row = cp.tile([1, 1024], bf16, tag="ones", name="ones")
            nc.vector.memset(ones_row[:], 1.0)
            # vext[m1, m0, hl, 0:64]=2v, [...,64]=2  (memset supplies the 2s)
            vext = cp.tile([128, 8, 2, 65], adt, tag="vext")
            nc.vector.memset(vext[:], 2.0)

            # preload the exp/tanh ACT table set during the input-DMA wait;
            # input is the just-memset ones row -> no DMA dependencies
            dum_t = actt.tile([128, 1024], f32, tag="silu_t", name="dum_t")
            nc.scalar.activation(dum_t[0:1, 0:64], ones_row[0:1, 0:64],
                                 AF.Tanh, scale=0.5)

            # ---- PE warmup on the ones row (no DMA dependency): flips the
            # HAM clock gate to 8/8 before the first real matmul ----
            wps = ps.tile([128, 1024], f32, tag="scp", name="warm")
            for wi in range(9):
                nc.tensor.matmul(
                    wps[:, 0:512], lhsT=ones_row[:, 0:128],
                    rhs=ones_row[:, 0:512],
                    start=(wi == 0), stop=(wi == 8))

            # conv epilogue: psum z already includes bias (K=1 bias matmul).
            # t = tanh(z/2); u = z*t; dst = z + u = z*(1+tanh(z/2)) = 2silu(z)
            def silu_epi(psz, dst_ap, tag, rr=None, a=4):
                t = actt.tile([128, 1024], f32, tag="silu_t", name=f"t_{tag}")
                u = actt.tile([128, 1024], f32, tag="silu_u", name=f"u_{tag}")
                w = psz.shape[-1]
                nc.scalar.activation(t[:, 0:w], psz, AF.Tanh, scale=0.5)
                nc.vector.tensor_mul(u[:, 0:w], psz, t[:, 0:w])
                psv, uv = psz, u[:, 0:w]
                if rr is not None:
                    psv = psv.rearrange(rr, a=a, h=2)
                    uv = uv.rearrange(rr, a=a, h=2)
                nc.vector.tensor_add(dst_ap, psv, uv)

            # ---- kv conv (k part), ONE [128,1024] psum:
            # kT[pp, m0, m1], pp = hl*64+d ----
            kT = cp.tile([128, 8, 128], adt, tag="kT")
            psk = ps.tile([128, 1024], f32, tag="scp", name="psk")
            for m0 in range(8):
                for c0 in range(2):
                    # PSUM accumulation groups are per 2KB bank: start=True
                    # on the first write to each of the two banks
                    nc.tensor.matmul(
                        psk[:, m0 * 128 : (m0 + 1) * 128],
                        lhsT=yk_sb[c0][:, m0 * 128 : (m0 + 1) * 128],
                        rhs=wkv_sb[c0],
                        start=(m0 % 4 == 0 and c0 == 0), stop=False)
            for hb in range(2):  # bias matmul per psum bank
                nc.tensor.matmul(
                    psk[:, hb * 512 : (hb + 1) * 512],
                    lhsT=ones_row[:, 0:128],
                    rhs=bkv8_sb[0:1, hb * 512 : (hb + 1) * 512],
                    start=False, stop=True)
            silu_epi(psk[:], kT[:].rearrange("p a b -> p (a b)"), "k")

            qT = cp.tile([128, 16, 256], adt, tag="qT")

            # paired q conv: covers (t2, t2+1) in one [128,1024] psum
            def q_conv_pair(t2):
                psq = ps.tile([128, 1024], f32, tag="scp", name=f"psq{t2}")
                for nn in range(4):
                    n0 = 2 * t2 + nn
                    for c0 in range(2):
                        nc.tensor.matmul(
                            psq[:, nn * 256 : (nn + 1) * 256],
                            lhsT=xq_sb[c0][:, n0 * 128 : (n0 + 1) * 128],
                            rhs=wq_sb[c0],
                            start=(nn % 2 == 0 and c0 == 0), stop=False)
                for hb in range(2):  # bias matmul per psum bank
                    nc.tensor.matmul(
                        psq[:, hb * 512 : (hb + 1) * 512],
                        lhsT=ones_row[:, 0:128],
                        rhs=bq4_sb[0:1, hb * 512 : (hb + 1) * 512],
                        start=False, stop=True)
                silu_epi(
                    psq[:],
                    qT[:, 2 * t2 : 2 * t2 + 4, :].rearrange("p a b -> p (a b)"),
                    f"q{t2}")

            q_conv_pair(0)  # t2 0-1

            # ---- kv conv (v part), ONE [128,1024] psum ----
            psv = ps.tile([128, 1024], f32, tag="scp", name="psv")
            for jv in range(2):
                for c0 in range(2):
                    nc.tensor.matmul(
                        psv[:, jv * 512 : (jv + 1) * 512],
                        lhsT=wkv_sb[c0],
                        rhs=yv_sb[c0][:, jv * 512 : (jv + 1) * 512],
                        start=(c0 == 0), stop=False)
            for hb in range(2):  # bias matmul per psum bank
                nc.tensor.matmul(
                    psv[:, hb * 512 : (hb + 1) * 512],
                    lhsT=bkvr_sb, rhs=ones_row[:, 0:512],
                    start=False, stop=True)
            silu_epi(psv[:], vext[:, 0:8, :, 0:64], "v",
                     rr="p (a h d) -> p a h d", a=8)

            # one [128,1024] score psum per m0 holds BOTH heads (hl0 cols
            # 0-511, hl1 cols 512-1023); the two matmuls target PE row
            # groups 0-63/64-127 and overlap in the array. The 3-deep scp
            # ring lets score production run up to 3 tiles ahead of exp.
            def scores_m0(t2, m0):
                scps = ps.tile([128, 1024], f32, tag="scp",
                               name=f"scp_{t2}_{m0}")
                for hl in range(2):
                    r0, r1 = hl * 64, (hl + 1) * 64
                    nc.tensor.matmul(
                        scps[:, hl * 512 : (hl + 1) * 512],
                        lhsT=kT[r0:r1, m0, :],
                        rhs=qT[r0:r1, 2 * t2 : 2 * t2 + 2, :],
                        start=True, stop=True)
                return scps

            # global score cursor: the first block's 8 tiles are created
            # before the loop (program order: exp(s) must follow scores(s));
            # the 3-deep scp ring throttles actual execution to ~3 ahead
            sc_tiles = {}
            sc_cursor = [0]

            def issue_next_score():
                s = sc_cursor[0]
                if s >= 64:
                    return
                sc_cursor[0] += 1
                sc_tiles[s] = scores_m0(s // 8, s % 8)

            for _ in range(8):
                issue_next_score()

            outun = [
                [cp.tile([128, 1024], adt, tag=f"outun{hl}_{i}",
                         name=f"outun{hl}_{i}") for i in range(2)]
                for hl in range(2)
            ]

            for t2 in range(8):
                att = []
                for m0 in range(8):
                    a = attp.tile([128, 1024], adt, tag="att",
                                  name=f"att_{t2}_{m0}")
                    # scoresT = 4*q.k ; want exp(q.k/8) -> scale 1/32
                    s = 8 * t2 + m0
                    nc.scalar.activation(
                        a[:], sc_tiles.pop(s)[:], AF.Exp, scale=0.03125)
                    att.append(a)

                # av: both heads interleaved by m0, chasing exp pairwise;
                # the next 8 score tiles are spliced between av pairs (the
                # scp ring throttles them to at most 3 ahead of exp)
                opss = {}
                for hl in range(2):
                    opss[hl] = ps.tile([128, 512], f32, tag="av", bufs=2,
                                       name=f"ops{hl}_{t2}")

                def av_pair(m0):
                    for hl in range(2):
                        nc.tensor.matmul(
                            opss[hl][0:65, :], lhsT=vext[:, m0, hl, :],
                            rhs=att[m0][:, hl * 512 : (hl + 1) * 512],
                            start=(m0 == 0), stop=(m0 == 7))

                av_pair(0)
                av_pair(1)
                issue_next_score()
                av_pair(2)
                issue_next_score()
                av_pair(3)
                # software-pipelined paired q conv fills the t2-end PE gap
                if t2 in (0, 2, 4):
                    q_conv_pair(t2 + 2)
                av_pair(4)
                issue_next_score()
                av_pair(5)
                issue_next_score()
                av_pair(6)
                issue_next_score()
                av_pair(7)
                issue_next_score()
                issue_next_score()
                issue_next_score()

                # normalize: denominators sit in psum row 64 (the 2.0-column
                # of V); gpsimd broadcasts them straight from PSUM, recip on
                # the [64,512] broadcast, multiply folded into the PSUM
                # evacuation; at odd t2 each head's projection follows its
                # normalize so proj(hl0) overlaps normalize(hl1)
                c0 = t2 % 2
                q0 = t2 // 2
                for hl in range(2):
                    ops = opss[hl]
                    # gpsimd can't read PSUM: hop the denominator row to SBUF
                    # (DVE; ScalarE at the very tail where it's idle), then
                    # ONE wide broadcast + recip on the broadcast
                    drow = gp.tile([1, 512], f32, tag="drow",
                                   name=f"drow{hl}_{t2}")
                    if t2 == 7 and hl == 0:
                        nc.scalar.copy(drow[:], ops[64:65, :])
                    else:
                        nc.vector.tensor_copy(drow[:], ops[64:65, :])
                    g = gp.tile([64, 512], f32, tag="g", name=f"g{hl}_{t2}")
                    rg = gp.tile([64, 512], f32, tag="rg", name=f"rg{hl}_{t2}")
                    nc.gpsimd.partition_broadcast(
                        g[0:64, :], drow[0:1, :], channels=64)
                    nc.vector.reciprocal_approx_fast(out=rg[:], in_=g[:])
                    for nn in range(2):
                        sl = slice(nn * 256, (nn + 1) * 256)
                        dst = outun[hl][c0][
                            nn * 64 : nn * 64 + 64,
                            q0 * 256 : (q0 + 1) * 256]
                        nc.vector.tensor_mul(dst, ops[0:64, sl], rg[0:64, sl])

                    if t2 % 2 == 0:
                        continue
                    # ---- quarter q0 complete for this head: projection ----
                    # proj fc in {2q0, 2q0+1}; rows hl*1024+half*512+q0+4r
                    ps2 = ps.tile([128, 512], f32, tag="av", bufs=2,
                                  name=f"psproj{hl}_{q0}")
                    for half in range(2):
                        fc = 2 * q0 + half
                        for cc in range(2):
                            nc.tensor.matmul(
                                ps2[:, half * 256 : (half + 1) * 256],
                                lhsT=outun[hl][cc][
                                    :, fc * 128 : (fc + 1) * 128],
                                rhs=wp_sb[cc],
                                start=(cc == 0), stop=(cc == 1))
                    osb = outp.tile([128, 512], bf16, tag="osb",
                                    name=f"osb{hl}_{q0}")
                    if q0 < 3:
                        nc.vector.tensor_add(osb[:], ps2[:], bp_bc[:])
                        dstap = bass.AP(
                            tensor=out,
                            offset=(hl * 1024 + q0) * 256,
                            ap=[[4 * 256, 128], [512 * 256, 2], [1, 256]])
                        nc.sync.dma_start(
                            dstap,
                            osb[:].rearrange("p (h c) -> p h c", h=2))
                    else:
                        # last quarter: per-half add+DMA on two queues so the
                        # final (tail-critical) output transfer is halved
                        for h in range(2):
                            sl = slice(h * 256, (h + 1) * 256)
                            nc.vector.tensor_add(
                                osb[:, sl], ps2[:, sl], bp_bc[:, sl])
                            dstap = bass.AP(
                                tensor=out,
                                offset=(hl * 1024 + q0 + h * 512) * 256,
                                ap=[[4 * 256, 128], [1, 256]])
                            eng = nc.sync if h == 0 else nc.scalar
                            eng.dma_start(dstap, osb[:, sl])

    nc.compile()
    return nc


def _prep_inputs(x, y, q_w, q_gamma, q_beta, q_mean, q_var,
                 kv_w, kv_gamma, kv_beta, kv_mean, kv_var, proj_w, proj_b):
    f = np.float32
    bf = ml_dtypes.bfloat16
    x = np.ascontiguousarray(np.asarray(x, f))
    y = np.ascontiguousarray(np.asarray(y, f))

    gq = np.asarray(q_gamma, f) / np.sqrt(np.asarray(q_var, f) + BN_EPS)
    bq_full = np.asarray(q_beta, f) - np.asarray(q_mean, f) * gq
    wq_host = np.ascontiguousarray((np.asarray(q_w, f) * gq[:, None]).T).astype(bf)

    gkv = np.asarray(kv_gamma, f) / np.sqrt(np.asarray(kv_var, f) + BN_EPS)
    bkv_full = np.asarray(kv_beta, f) - np.asarray(kv_mean, f) * gkv
    wkv_host = np.ascontiguousarray((np.asarray(kv_w, f) * gkv[:, None]).T).astype(bf)

    wp_host = np.ascontiguousarray(np.asarray(proj_w, f).T).astype(bf)
    bp_host = np.asarray(proj_b, f)

    bq4 = np.tile(bq_full[None, :], (1, 4)).astype(bf)
    bkv8_h = np.tile(bkv_full[None, :], (1, 8)).astype(bf)
    bkvr_h = bkv_full[None, :].astype(bf)
    bp2 = np.tile(bp_host[None, :], (1, 2)).astype(f)

    def fold(a):
        # [256, W] -> [128, 2W]: row halves side by side (one 2D DMA)
        return np.ascontiguousarray(
            np.concatenate([a[:128], a[128:]], axis=1))

    biases = np.concatenate([bq4, bkv8_h, bkvr_h], axis=1).astype(bf)
    wq_f = fold(wq_host)
    wkv_f = fold(wkv_host)
    wp_f = fold(wp_host)

    in_maps = []
    for core in range(8):
        b, hp = core // 2, core % 2
        X4 = x[b].reshape(C, N_TOK)
        Y4 = y[b].reshape(C, N_TOK)
        xqa = np.ascontiguousarray(
            X4.reshape(C, 16, 256)[:, :, hp * 128 : (hp + 1) * 128]
        ).reshape(C, 2048).astype(bf)
        Y8 = Y4.reshape(C, 8, 512)
        yka = np.ascontiguousarray(
            Y8[:, :, hp * 128 : (hp + 1) * 128]).reshape(C, 1024).astype(bf)
        yva = np.ascontiguousarray(
            Y8[:, :, 256 + hp * 128 : 256 + (hp + 1) * 128]
        ).reshape(C, 1024).astype(bf)
        in_maps.append({
            "xq": fold(xqa), "yk": fold(yka), "yv": fold(yva),
            "wq": wq_f, "wkv": wkv_f, "wp": wp_f,
            "biases": biases, "bp": bp2,
        })
    return in_maps


def _get_nc():
    if "nc" not in _CACHE:
        _CACHE["nc"] = _build()
    return _CACHE["nc"]


def kernel(x, y, H=64, W=64, q_w=None, q_gamma=None, q_beta=None, q_mean=None,
           q_var=None, kv_w=None, kv_gamma=None, kv_beta=None, kv_mean=None,
           kv_var=None, proj_w=None, proj_b=None, _trace=False):
    from concourse.bass_utils import run_bass_kernel_spmd

    nc = _get_nc()
    in_maps = _prep_inputs(x, y, q_w, q_gamma, q_beta, q_mean, q_var,
                           kv_w, kv_gamma, kv_beta, kv_mean, kv_var,
                           proj_w, proj_b)
    kw = {}
    if _trace:
        kw = {"trace": True, "trace_cores": list(range(8))}
    res = run_bass_kernel_spmd(nc, in_maps, list(range(8)), **kw)
    outa = np.empty((B, N_TOK, C), np.float32)
    for core in range(8):
        b, hp = core // 2, core % 2
        outa[b, hp * 2048 : (hp + 1) * 2048, :] = res.results[core]["out"]
    if _trace:
        return outa, res
    return outa
